# revision 55
# baseline (speedup 1.0000x reference)
"""MLA forward Bass kernel for 8 TRN2 NeuronCores.

Sharding: pure query-row sharding. Core c handles batch b = c//4 and query
rows [sl*512, (sl+1)*512) with sl = c%4, for ALL 16 heads. The host ROTATES
each core's sequence (and rope tables) by sl*512 so the core's own query
block is always columns 0:SQ -- the SPMD program is position-independent and
softmax over keys is permutation-invariant. Keys/values span the full
sequence, so the compressed-KV path is computed per-core for the whole batch
(replicated across the 4 cores sharing a batch; a DRAM AllGather costs ~90us
on this axon path and loses). The output projection contracts all 16 heads
locally, so the host just concatenates the 8 row-blocks.

Numerics: everything bf16 except the ATTENTION SCORES, which run as fp8e4
DoubleRow matmuls -- one K=256 MM per (head, kt) with nope dims in
contraction slot i=0 and the pair's rope dims stacked in i=1 (rows 0:64 even
head / 64:128 odd head, zeroed on the k side for the other head). Score
operand quantization is the only fp8 the 2e-2 rel_fro budget tolerates
(logit errors shrink by SCALE before exp): measured 1.696e-2 total vs 5.1e-3
all-bf16; every other stage quantized alone busts the budget (see
quant_study.py, which matches HW to 3 digits).

Structure (measured ~600us traced vs 898us for the pre-fp8 baseline):
- Phase 0: x is held in SBUF as 16 full-row [128, S] tiles (one 4KB-line DMA
  each, own-block columns first) serving all 4 kv blocks AND the q compress.
  rmsnorm applies as broadcast-std ones-matmul -> wide DVE reciprocal ->
  one fused (raw*g)*(1/std) scalar_tensor_tensor per chunk. PSUM drains
  alternate ACT/DVE so block-boundary ring stalls halve.
- Phase A is software-pipelined over head pairs: prep_chunks(p+1) (k/q
  decompress MMs + fp8 drains + DVE rotate-half + group-start v decompress)
  is emitted in chunks INSIDE pair p's kt loop, so the in-order PE queue
  always has filler ahead of each exp-gated score matmul. Both heads'
  scores land in one [128,1024] two-bank PSUM tile -> single wide exp
  (ACT (N+352)/1.2ns per instr; fusing halves the overhead) and two
  alternating DVE den chains. Rotate-half runs on DVE via +-32-partition
  swapped muls against a sign-folded/swapped sin table (sinqsB) -- DVE
  tensor-tensor requires equal input base partitions.
- Softmax normalization (deferred one pair) uses fast-ACT-drained ones-
  matmuls for reduce/broadcast and the wide [128,SQ] DVE reciprocal on SBUF
  (a [1,SQ] reciprocal runs on one DVE lane, 3.3us vs 1.3us wide).
- Phase B: h-outer over two D-halves, wo streamed as [128,1024] row tiles
  (2KB DMA lines), 8 PSUM accumulators per half, oT stationary loaded once
  per (half, qt).
- PSUM budget is exactly 8 banks in phase A: st(2x2) + ot(2) + wk(2); the
  p0-only aux pool is closed before phase A starts.
Pitfalls hit: gpsimd custom ISA (partition_all_reduce) fails codegen ("ISA
wrong length"); gpsimd DVE-style tensor ops run ~3us per [128,1024] (too
slow); DVE tensor-tensor rejects mismatched input base partitions; a
[1,512] DVE reciprocal is ~3.3us. Device clock: PE 2.0GHz (P0), HAM duty-
throttles ~40us bursts at half clock under sustained load (11-21% of span,
grows when PE idles -- density begets density). ~±4% run-to-run noise.
"""

import os
import sys

for _p in ("/root/.axon_site/_ro/trn_rl_repo", "/opt/trn_rl_repo"):
    if os.path.isdir(_p) and _p not in sys.path:
        sys.path.insert(0, _p)

import numpy as np

import concourse.bass as bass
import concourse.tile as tile
from concourse import mybir
from concourse.bass_utils import run_bass_kernel_spmd

F32 = mybir.dt.float32
F32R = mybir.dt.float32r
BF16 = mybir.dt.bfloat16
F8 = mybir.dt.float8e4
DR = mybir.MatmulPerfMode.DoubleRow

D = 2048        # d_model
S = 2048        # seq len
B = 2           # batch
H = 16          # heads
HD = 128        # nope head dim
KV = 512        # kv lora rank
QL = 768        # q lora rank
RD = 64         # rope dim
EPS = 1e-6
SQ = 512        # query rows per core
N_CORES = 8
GROUPS = 4      # head groups of 4
GH = 4          # heads per group
SCALE = 1.0 / float(np.sqrt(HD + RD))

NKV = KV // 128   # 4 kv-lora chunks
NQL = QL // 128   # 6 q-lora chunks
NS = S // 512     # 4 seq blocks
NST = S // 128    # 16 seq tiles


# ---------------------------------------------------------------------------
# The walrus build in this container only encodes a single sync-wait on a
# Drain (TPB_CTRL) instruction, but TileContext._drain_and_barrier parks the
# whole global-clock wait set on the tail drain ("Too many sync wait
# commands"). Hoist the waits onto single-wait NOPs ahead of a bare drain.
def _patch_tile_drain():
    from bass_rust import ScopedClock

    def _drain_and_barrier(self, tick_clock, wait_clock):
        probe = self.nc.sync.nop(nofuse=True)
        wait_clock.add_sem_waits(
            probe.ins, ScopedClock({None: tick_clock.global_clock})
        )
        si = probe.ins.sync_info
        waits = list(si.on_wait) if si is not None else []
        if len(waits) > 1:
            probe.ins.sync_info = mybir.SyncInfo(on_wait=waits[:1], on_update=[])
            for w in waits[1:]:
                extra = self.nc.sync.nop(nofuse=True)
                extra.ins.sync_info = mybir.SyncInfo(on_wait=[w], on_update=[])
        self.nc.sync.drain()

        self.nc.all_engine_barrier()
        assert self.sems is not None
        popped = self.nc._tile_sem_poison_stack.pop()
        assert popped is self._sem_poison
        self.nc.clear_and_free_semaphores(list(self.sems.allocated().values()))
        self.nc.all_engine_barrier()

    tile.TileContext._drain_and_barrier = _drain_and_barrier


_patch_tile_drain()


def _r(ap):
    return ap.bitcast(F32R)


def build_nc():
    nc = bass.Bass()

    xT = nc.dram_tensor("xT", [D, S], BF16, kind="ExternalInput")
    wcq = nc.dram_tensor("wcq", [D, QL], BF16, kind="ExternalInput")
    wckv = nc.dram_tensor("wckv", [D, KV], BF16, kind="ExternalInput")
    wkr2 = nc.dram_tensor("wkr2", [D, 128], BF16, kind="ExternalInput")
    wdq = nc.dram_tensor("wdq", [QL, H * HD], BF16, kind="ExternalInput")
    wdqr = nc.dram_tensor("wdqr", [QL, H * RD], BF16, kind="ExternalInput")
    wdk = nc.dram_tensor("wdk", [KV, H * HD], BF16, kind="ExternalInput")
    wdv = nc.dram_tensor("wdv", [KV, H * HD], BF16, kind="ExternalInput")
    wo = nc.dram_tensor("wo", [H * HD, D], BF16, kind="ExternalInput")
    gqT = nc.dram_tensor("gqT", [128, NQL], F32, kind="ExternalInput")
    gkvT = nc.dram_tensor("gkvT", [128, NKV], F32, kind="ExternalInput")
    cosk = nc.dram_tensor("cosk", [128, S], F32, kind="ExternalInput")
    sink = nc.dram_tensor("sink", [128, S], F32, kind="ExternalInput")
    rotp = nc.dram_tensor("rotp", [128, 128], BF16, kind="ExternalInput")
    out = nc.dram_tensor("out", [SQ, D], F32, kind="ExternalOutput")
    debug = bool(int(os.environ.get("MLA_DEBUG", "0")))
    if debug:
        dbg_kvcT = nc.dram_tensor("dbg_kvcT", [KV, S], BF16, kind="ExternalOutput")
        dbg_qcT = nc.dram_tensor("dbg_qcT", [QL, SQ], BF16, kind="ExternalOutput")
        dbg_krT = nc.dram_tensor("dbg_krT", [128, S], BF16, kind="ExternalOutput")
        dbg_oT = nc.dram_tensor("dbg_oT", [H * 128, SQ], BF16, kind="ExternalOutput")

    with tile.TileContext(nc) as tc:
        _build_body(nc, tc, locals(), debug)
    _split_excess_waits(nc)
    return nc


# This walrus build encodes at most one sync-wait per engine instruction;
# hoist surplus waits onto single-wait NOPs right before the instruction on
# the same engine queue (in-order execution keeps the semantics identical).
def _split_excess_waits(nc, max_waits=1):
    n_nops = 0
    for f in nc.m.functions:
        for bb in f.blocks:
            out = []
            for ins in bb.instructions:
                si = ins.sync_info
                if si is not None:
                    sem = [w for w in si.on_wait if w.sync_type == "semaphore"]
                    other = [w for w in si.on_wait if w.sync_type != "semaphore"]
                    budget = max(max_waits - len(other), 0)
                    if len(sem) > budget:
                        extra, keep = sem[:-budget] if budget else sem, (
                            sem[-budget:] if budget else [])
                        for j, w in enumerate(extra):
                            nop = mybir.InstNoOp(
                                name=f"{ins.name}-wsplit{j}",
                                engine=ins.engine,
                                bass_nofuse=True,
                                sync_info=mybir.SyncInfo(
                                    on_wait=[w], on_update=[]),
                            )
                            out.append(nop)
                            n_nops += 1
                        ins.sync_info = mybir.SyncInfo(
                            on_wait=other + keep,
                            on_update=list(si.on_update))
                out.append(ins)
            bb.instructions = out
    return n_nops


def _norm_den(nc, misc, st_ps, ones128, pending):
    """Stage 1 of softmax normalization: per head, den1 = sum_partitions of
    the two den partials via two accumulating ones-matmuls, drained FAST by
    an ACT copy so the st slot frees immediately (no slow DVE tenant)."""
    d1s = []
    for hA, hB, potA, potB, den_e, den_o in pending:
        for sl in (slice(0, SQ), slice(SQ, 2 * SQ)):
            dst = st_ps.tile([128, SQ], F32, tag="st", name="dst")
            nc.tensor.matmul(dst[0:1, :], _r(ones128), _r(den_e[:, sl]),
                             start=True, stop=False)
            nc.tensor.matmul(dst[0:1, :], _r(ones128), _r(den_o[:, sl]),
                             start=False, stop=True)
            d1 = misc.tile([1, SQ], F32R, tag="rec")
            nc.scalar.copy(d1, dst[0:1, :])
            d1s.append(d1)
    return d1s


def _norm_fin(nc, misc, st_ps, oT, ones1, pending, d1s):
    """Stage 2: broadcast den across partitions (ones-matmul, drained fast
    by ACT), then the wide [128, SQ] DVE reciprocal on SBUF, then scale."""
    i = 0
    for hA, hB, potA, potB, den_e, den_o in pending:
        for h, pot in ((hA, potA), (hB, potB)):
            d1 = d1s[i]
            i += 1
            bc = st_ps.tile([128, SQ], F32, tag="st", name="bcst")
            nc.tensor.matmul(bc, _r(ones1), _r(d1))
            bc_s = misc.tile([128, SQ], F32R, tag="bcs")
            nc.scalar.copy(bc_s, bc)
            rec = misc.tile([128, SQ], F32R, tag="recn")
            with nc.allow_low_precision(reason="f32r is full fp32 bits"):
                nc.vector.reciprocal(rec, bc_s)
            nc.vector.tensor_mul(oT[h], _r(pot), rec)


def _build_body(nc, tc, t, debug=False):
    from contextlib import ExitStack

    ctx = ExitStack()
    with ctx:
        consts = ctx.enter_context(tc.tile_pool(name="consts", bufs=1))
        persist = ctx.enter_context(tc.tile_pool(name="persist", bufs=1))
        misc = ctx.enter_context(tc.tile_pool(name="misc", bufs=2))
        # PSUM pools: aux lives through phases 0+A, closed before phase B
        # (which needs all 8 banks for its accumulators).
        aux_ctx = ExitStack()
        aux_ps = aux_ctx.enter_context(
            tc.tile_pool(name="aux_ps", bufs=1, space="PSUM"))

        # ---- constants -----------------------------------------------------
        ones128f = consts.tile([128, 1], F32)
        nc.vector.memset(ones128f, 1.0)
        ones128 = consts.tile([128, 1], F32R)
        nc.scalar.copy(ones128, ones128f)
        ones1f = consts.tile([1, 128], F32)
        nc.vector.memset(ones1f, 1.0)
        ones1 = consts.tile([1, 128], F32R)
        nc.scalar.copy(ones1, ones1f)
        gqT_s = consts.tile([128, NQL], F32)
        nc.sync.dma_start(out=gqT_s, in_=t["gqT"][:, :])
        gkvT_s = consts.tile([128, NKV], F32)
        nc.sync.dma_start(out=gkvT_s, in_=t["gkvT"][:, :])
        eps_s = consts.tile([1, 1], F32)
        nc.vector.memset(eps_s, EPS)
        rotp_s = consts.tile([128, 128], BF16)
        nc.sync.dma_start(out=rotp_s, in_=t["rotp"][:, :])
        # the query rows are always the first SQ columns of the (per-core
        # rotated) sequence, so the q rope tables are slices of the k ones.
        # (tiles allocated here; DMAs emitted at phase A start so the
        # prologue DMA queue serves the compress inputs first)
        cosq_s = consts.tile([128, SQ], F32)
        sinq_s = consts.tile([128, SQ], F32)
        sinqsB = consts.tile([128, SQ], F32)

        # ---- persistent tiles (bf16: matmul operands -> FWL weight loads) --
        kvcT = [persist.tile([128, S], BF16, tag=f"kvcT{c}", name=f"kvcT{c}") for c in range(NKV)]
        krT = persist.tile([128, S], BF16, tag="krT")
        qcT = [persist.tile([128, SQ], BF16, tag=f"qcT{c}", name=f"qcT{c}") for c in range(NQL)]
        oT = [persist.tile([128, SQ], BF16, tag=f"oT{h}", name=f"oT{h}") for h in range(H)]

        # ===================================================================
        # Phase 0: compress. kvcT/krT over full seq, qcT over own query rows.
        # ===================================================================
        with nc.named_scope("p0_compress", notify=True), \
             tc.tile_pool(name="misc0", bufs=2) as misc0, \
             tc.tile_pool(name="xhold", bufs=1) as xholdp, \
             tc.tile_pool(name="wkvhold", bufs=1) as wkvhold, \
             tc.tile_pool(name="wstream", bufs=6) as wstream, \
             tc.tile_pool(name="acc_ps", bufs=7, space="PSUM") as acc_ps:
            # wckv/wkr are reused by all 4 seq blocks: load once, keep in SBUF
            wkv_h = [wkvhold.tile([128, KV], BF16, tag=f"wckv{d}", name=f"wckv{d}")
                     for d in range(16)]
            wkr_h = [wkvhold.tile([128, 128], BF16, tag=f"wkr{d}", name=f"wkr{d}")
                     for d in range(16)]
            # x held in SBUF as 16 full-row tiles: one 4KB-contiguous-line DMA
            # per d-chunk serves all 4 kv blocks AND the q compress (the query
            # block is columns 0:SQ of the rotated sequence).
            xh = [xholdp.tile([128, S], BF16, tag=f"xh{d}", name=f"xh{d}")
                  for d in range(16)]
            # block 0 only reads columns 0:512, so those stream first and
            # the first matmul starts ~4x sooner; the rest follows, then the
            # wcq tiles (needed only by the q compress ~100us later).
            for d in range(16):
                drow = slice(d * 128, (d + 1) * 128)
                nc.sync.dma_start(out=wkv_h[d], in_=t["wckv"][drow, :])
                nc.sync.dma_start(out=wkr_h[d], in_=t["wkr2"][drow, :])
                nc.sync.dma_start(out=xh[d][:, 0:512], in_=t["xT"][drow, 0:512])
            for d in range(16):
                drow = slice(d * 128, (d + 1) * 128)
                nc.sync.dma_start(out=xh[d][:, 512:S], in_=t["xT"][drow, 512:S])
            wq_t = [wstream.tile([128, QL], BF16, tag="wcq", name="wq_t")
                    for _ in range(16)]
            for d in range(16):
                drow = slice(d * 128, (d + 1) * 128)
                nc.sync.dma_start(out=wq_t[d], in_=t["wcq"][drow, :])
            def p0_post(scol, kvraw, kraw):
                # rmsnorm over kv features (partition dim across the 4 chunks)
                ssq = aux_ps.tile([1, 512], F32, tag="aux")
                for c in range(NKV):
                    sq = misc0.tile([128, 512], F32R, tag="sq")
                    nc.vector.tensor_mul(sq, kvraw[c], kvraw[c])
                    nc.tensor.matmul(ssq, _r(ones128), _r(sq),
                                     start=(c == 0), stop=(c == NKV - 1))
                std1 = misc0.tile([1, 512], F32R, tag="rstd")
                nc.scalar.activation(std1, ssq,
                                     mybir.ActivationFunctionType.Sqrt,
                                     bias=eps_s[:, :], scale=1.0 / KV)
                # broadcast std across partitions FIRST (one ones-matmul),
                # reciprocal on the wide tile (a [1,512] DVE reciprocal runs
                # on a single lane ~3.3us), then one fused
                # (kvraw * g) * (1/std) DVE op per chunk.
                bc0 = aux_ps.tile([128, 512], F32, tag="aux")
                nc.tensor.matmul(bc0, _r(ones1), _r(std1))
                rec = misc0.tile([128, 512], F32R, tag="recw")
                with nc.allow_low_precision(reason="f32r is full fp32 bits"):
                    nc.vector.reciprocal(rec, bc0.bitcast(F32R))
                for c in range(NKV):
                    nc.vector.scalar_tensor_tensor(
                        kvcT[c][:, scol], kvraw[c], gkvT_s[:, c:c + 1], rec,
                        mybir.AluOpType.mult, mybir.AluOpType.mult)

                # rope on the (duplicated-rows) k_rope block
                ck = misc0.tile([128, 512], F32, tag="ck")
                nc.sync.dma_start(out=ck, in_=t["cosk"][:, scol])
                sk = misc0.tile([128, 512], F32, tag="sk")
                nc.sync.dma_start(out=sk, in_=t["sink"][:, scol])
                rot = aux_ps.tile([128, 512], F32, tag="aux")
                nc.tensor.matmul(rot, rotp_s, kraw)
                t1 = misc0.tile([128, 512], F32, tag="ropet1")
                nc.vector.tensor_mul(t1, kraw, ck)
                t2 = misc0.tile([128, 512], F32, tag="ropet2")
                nc.vector.tensor_mul(t2, rot, sk)
                nc.vector.tensor_add(krT[:, scol], t1, t2)

            p0_pending = None
            for sb in range(NS):
                scol = slice(sb * 512, (sb + 1) * 512)
                pkv = [acc_ps.tile([128, 512], F32, tag="acc", name="pkv") for _ in range(NKV)]
                pkr = acc_ps.tile([128, 512], F32, tag="acc")
                for d in range(16):
                    for c in range(NKV):
                        nc.tensor.matmul(
                            pkv[c], wkv_h[d][:, c * 128:(c + 1) * 128],
                            xh[d][:, scol], start=(d == 0), stop=(d == 15))
                    nc.tensor.matmul(pkr, wkr_h[d], xh[d][:, scol],
                                     start=(d == 0), stop=(d == 15))
                # drain psum to raw bf16 sbuf tiles (releases acc banks), then
                # run the PREVIOUS block's normalize behind this block's MMs.
                kvraw = [misc0.tile([128, 512], BF16, tag=f"kvraw{c}",
                                    name=f"kvraw{c}")
                         for c in range(NKV)]
                for c in range(NKV):
                    if c % 2 == 0:
                        nc.scalar.copy(kvraw[c], pkv[c])
                    else:
                        nc.vector.tensor_copy(kvraw[c], pkv[c])
                kraw = misc0.tile([128, 512], BF16, tag="kraw")
                nc.vector.tensor_copy(kraw, pkr)
                if p0_pending is not None:
                    p0_post(*p0_pending)
                p0_pending = (scol, kvraw, kraw)

            # qcT over own query rows (= columns 0:SQ of the rotated seq)
            pqc = [acc_ps.tile([128, 512], F32, tag="acc", name="pqc") for _ in range(NQL)]
            for d in range(16):
                for c in range(NQL):
                    nc.tensor.matmul(
                        pqc[c], wq_t[d][:, c * 128:(c + 1) * 128],
                        xh[d][:, 0:SQ], start=(d == 0), stop=(d == 15))
            if p0_pending is not None:
                p0_post(*p0_pending)
                p0_pending = None
            ssq = aux_ps.tile([1, 512], F32, tag="aux")
            for c in range(NQL):
                sq = misc0.tile([128, 512], F32R, tag="sq")
                nc.scalar.square(sq, pqc[c])
                nc.tensor.matmul(ssq, _r(ones128), _r(sq),
                                 start=(c == 0), stop=(c == NQL - 1))
            std1 = misc0.tile([1, 512], F32R, tag="rstd")
            nc.scalar.activation(std1, ssq, mybir.ActivationFunctionType.Sqrt,
                                 bias=eps_s[:, :], scale=1.0 / QL)
            bc0 = aux_ps.tile([128, 512], F32, tag="aux")
            nc.tensor.matmul(bc0, _r(ones1), _r(std1))
            rec = misc0.tile([128, 512], F32R, tag="recw")
            with nc.allow_low_precision(reason="f32r is full fp32 bits"):
                nc.vector.reciprocal(rec, bc0.bitcast(F32R))
            for c in range(NQL):
                nc.vector.scalar_tensor_tensor(
                    qcT[c], pqc[c], gqT_s[:, c:c + 1], rec,
                    mybir.AluOpType.mult, mybir.AluOpType.mult)

        # aux_ps is only used by phase 0; release its PSUM bank so phase A
        # can run the fused [128,1024] score tiles within the 8-bank budget.
        aux_ctx.close()

        # ===================================================================
        # Phase A: per head group -- decompress k/v/q, attention.
        # Scores run as fp8e4 DoubleRow matmuls: contraction slots [p, i]
        # hold nope dims (i=0) and rope dims (i=1, rows 0:64 for even heads /
        # 64:128 for odd heads, zero elsewhere), so one K=256 DR matmul per
        # (head, kt) replaces the K=128 nope + K=64 rope pair. fp8 on the
        # score operands costs ~0.9% rel err on the output (logit errors are
        # shrunk by SCALE before exp; measured in quant_study.py).
        # ===================================================================
        with nc.named_scope("pA_attn", notify=True), \
             tc.tile_pool(name="vpool", bufs=32) as vpool, \
             tc.tile_pool(name="khp", bufs=4) as khp, \
             tc.tile_pool(name="qmp", bufs=4) as qmp, \
             tc.tile_pool(name="ptp", bufs=6) as ptp, \
             tc.tile_pool(name="otsp", bufs=4) as otsp, \
             tc.tile_pool(name="denp", bufs=6) as denp, \
             tc.tile_pool(name="wdqp", bufs=12) as wdqp, \
             tc.tile_pool(name="wdqrp", bufs=12) as wdqrp, \
             tc.tile_pool(name="wdkp", bufs=8) as wdkp, \
             tc.tile_pool(name="wdvp", bufs=8) as wdvp, \
             tc.tile_pool(name="st_ps", bufs=2, space="PSUM") as st_ps, \
             tc.tile_pool(name="ot_ps", bufs=2, space="PSUM") as ot_ps, \
             tc.tile_pool(name="wk_ps", bufs=2, space="PSUM") as wk_ps:

            # kr-with-zeros fp8 patterns DMAd into each pair's kh[:, 1, :]:
            # krzA rows 0:64 = kr (even head), krzB rows 64:128 = kr (odd).
            nc.sync.dma_start(out=cosq_s, in_=t["cosk"][:, 0:SQ])
            nc.sync.dma_start(out=sinq_s, in_=t["sink"][:, 0:SQ])
            # Swapped+sign-folded sin table for the DVE-only rotate-half:
            # t2[r] = rot(q)[r]*sin[r] with rot(q)[r] = -q[r+32] / +q[r-32]
            # (within each 64-row half). Each strided mul reads q[swap(r)]
            # and sinqsB[swap(r)] from the SAME partitions (DVE requires
            # equal input base partitions): sinqsB[i+32] = -sin[i],
            # sinqsB[i] = +sin[i+32].
            nc.vector.tensor_scalar_mul(sinqsB[32:64, :], sinq_s[0:32, :], -1.0)
            nc.vector.tensor_copy(sinqsB[0:32, :], sinq_s[32:64, :])
            nc.vector.tensor_scalar_mul(sinqsB[96:128, :], sinq_s[64:96, :], -1.0)
            nc.vector.tensor_copy(sinqsB[64:96, :], sinq_s[96:128, :])
            krzA = persist.tile([128, S], F8, tag="krzA")
            krzB = persist.tile([128, S], F8, tag="krzB")
            nc.vector.memset(krzA, 0.0)
            nc.vector.memset(krzB, 0.0)
            nc.scalar.copy(krzA[0:64, :], krT[0:64, :])
            nc.scalar.copy(krzB[64:128, :], krT[64:128, :])

            pending = []
            state = {}
            NPAIRS = GROUPS * (GH // 2)

            def prep_chunks(p):
                """Generator emitting pair p's decompress work in chunks;
                consumed from inside pair p-1's kt loop so the in-order PE
                queue always has filler ahead of each exp-gated score
                matmul. Group starts also emit the group's weight DMAs and
                the shared v decompress."""
                g, pair = divmod(p, GH // 2)
                gcol = slice(g * 512, (g + 1) * 512)
                if pair == 0:
                    wdv_t = [wdvp.tile([128, 512], BF16, tag="wdv", name="wdv_t")
                             for _ in range(NKV)]
                    for c in range(NKV):
                        nc.sync.dma_start(
                            out=wdv_t[c], in_=t["wdv"][c * 128:(c + 1) * 128, gcol])
                    wdk_t = [wdkp.tile([128, 512], BF16, tag="wdk", name="wdk_t")
                             for _ in range(NKV)]
                    for c in range(NKV):
                        nc.sync.dma_start(
                            out=wdk_t[c], in_=t["wdk"][c * 128:(c + 1) * 128, gcol])
                    wdq_t = [wdqp.tile([128, 512], BF16, tag="wdq", name="wdq_t")
                             for _ in range(NQL)]
                    for c in range(NQL):
                        nc.sync.dma_start(
                            out=wdq_t[c], in_=t["wdq"][c * 128:(c + 1) * 128, gcol])
                    grcol = slice(g * 256, (g + 1) * 256)
                    wdqr_t = [wdqrp.tile([128, 256], BF16, tag="wdqr", name="wdqr_t")
                              for _ in range(NQL)]
                    for c in range(NQL):
                        nc.sync.dma_start(
                            out=wdqr_t[c], in_=t["wdqr"][c * 128:(c + 1) * 128, grcol])
                    state[("w", g)] = (wdv_t, wdk_t, wdq_t, wdqr_t)
                    yield 1
                    # v for all 4 heads of the group: moving = wdv (512
                    # wide), stationary = kvc seq-tile; drained on DVE (ACT
                    # is exp-saturated in the kt loops this interleaves with)
                    vt = {}
                    for st in range(NST):
                        pv = wk_ps.tile([128, 512], F32, tag="wk")
                        for c in range(NKV):
                            nc.tensor.matmul(
                                pv, kvcT[c][:, st * 128:(st + 1) * 128], wdv_t[c],
                                start=(c == 0), stop=(c == NKV - 1))
                        v_s = vpool.tile([128, 512], BF16, tag="v")
                        nc.vector.tensor_copy(v_s, pv)
                        vt[st] = v_s
                        if st % 2 == 1:
                            yield 1
                    state[("v", g)] = vt
                wdv_t, wdk_t, wdq_t, wdqr_t = state[("w", g)]
                colA = slice((2 * pair) * 128, (2 * pair + 1) * 128)
                colB = slice((2 * pair + 1) * 128, (2 * pair + 2) * 128)

                # k^T DoubleRow tiles for both heads: [128, 2, S] fp8
                # (i=0 nope from decompress, i=1 rope pattern via DMA)
                khA = khp.tile([128, 2, S], F8, tag="kh", name="khA")
                khB = khp.tile([128, 2, S], F8, tag="kh", name="khB")
                nc.sync.dma_start(out=khA[:, 1, :], in_=krzA[:, :])
                nc.sync.dma_start(out=khB[:, 1, :], in_=krzB[:, :])
                for kh, hcol in ((khA, colA), (khB, colB)):
                    for blk in range(NS):
                        bcol = slice(blk * 512, (blk + 1) * 512)
                        pk = wk_ps.tile([128, 512], F32, tag="wk")
                        for c in range(NKV):
                            nc.tensor.matmul(
                                pk, wdk_t[c][:, hcol], kvcT[c][:, bcol],
                                start=(c == 0), stop=(c == NKV - 1))
                        nc.scalar.copy(kh[:, 0, bcol], pk)
                        yield 1

                # q DoubleRow tiles for both heads: [128, 2, SQ] fp8
                qmA = qmp.tile([128, 2, SQ], F8, tag="qm", name="qmA")
                qmB = qmp.tile([128, 2, SQ], F8, tag="qm", name="qmB")
                for qm, hcol in ((qmA, colA), (qmB, colB)):
                    pq = wk_ps.tile([128, SQ], F32, tag="wk")
                    for c in range(NQL):
                        nc.tensor.matmul(pq, wdq_t[c][:, hcol], qcT[c],
                                         start=(c == 0), stop=(c == NQL - 1))
                    nc.scalar.copy(qm[:, 0, :], pq)
                    yield 1

                # q_rope for the pair (two heads stacked on partitions)
                prcol = slice(pair * 128, (pair + 1) * 128)
                pqr = wk_ps.tile([128, SQ], F32, tag="wk")
                for c in range(NQL):
                    nc.tensor.matmul(
                        pqr, wdqr_t[c][:, prcol], qcT[c],
                        start=(c == 0), stop=(c == NQL - 1))
                qraw = misc.tile([128, SQ], BF16, tag="qraw")
                nc.scalar.copy(qraw, pqr)
                yield 1
                # rotate-half entirely on DVE: +-32 partition-swapped muls
                # against the sign-folded-and-swapped sin table
                t1 = misc.tile([128, SQ], F32, tag="ropet1")
                nc.vector.tensor_mul(t1, qraw, cosq_s)
                t2 = misc.tile([128, SQ], F32, tag="ropet2")
                nc.vector.tensor_mul(t2[0:32, :], qraw[32:64, :],
                                     sinqsB[32:64, :])
                nc.vector.tensor_mul(t2[32:64, :], qraw[0:32, :],
                                     sinqsB[0:32, :])
                nc.vector.tensor_mul(t2[64:96, :], qraw[96:128, :],
                                     sinqsB[96:128, :])
                nc.vector.tensor_mul(t2[96:128, :], qraw[64:96, :],
                                     sinqsB[64:96, :])
                nc.vector.tensor_add(qmA[:, 1, :], t1, t2)
                nc.vector.tensor_add(qmB[:, 1, :], t1, t2)
                state[p] = (khA, khB, qmA, qmB)

            # prologue: pair 0's prep runs un-overlapped
            for _ in prep_chunks(0):
                pass

            for p in range(NPAIRS):
                g, pair = divmod(p, GH // 2)
                hA = g * GH + 2 * pair
                hB = hA + 1
                colA = slice((2 * pair) * 128, (2 * pair + 1) * 128)
                colB = slice((2 * pair + 1) * 128, (2 * pair + 2) * 128)
                khA, khB, qmA, qmB = state.pop(p)
                vt = state[("v", g)]

                gen = prep_chunks(p + 1) if p + 1 < NPAIRS else None
                if gen is not None:
                    for _ in range(5):
                        if next(gen, None) is None:
                            gen = None
                            break

                # normalization of the previous pair, emitted BEHIND a few
                # prep chunks so its den-tail wait has PE filler in front
                d1s = _norm_den(nc, misc, st_ps, ones128, pending)
                _norm_fin(nc, misc, st_ps, oT, ones1, pending, d1s)
                pending.clear()

                # attention for the pair: one fp8 DoubleRow matmul per
                # (head, kt), both heads' scores in one [128,1024] two-bank
                # PSUM tile -> single wide exp + den ops. Up to two prep
                # chunks of pair p+1 are emitted per kt so the PE queue has
                # filler in front of each exp-gated score matmul.
                potA = ot_ps.tile([128, SQ], F32, tag="ot")
                potB = ot_ps.tile([128, SQ], F32, tag="ot")
                den_e = denp.tile([128, 2 * SQ], F32R, tag="den")
                den_o = denp.tile([128, 2 * SQ], F32R, tag="den")
                for kt in range(NST):
                    kcol = slice(kt * 128, (kt + 1) * 128)
                    pst2 = st_ps.tile([128, 2 * SQ], F32, tag="st")
                    nc.tensor.matmul(pst2[:, 0:SQ], khA[:, :, kcol],
                                     qmA[:, :, :],
                                     perf_mode=DR, start=True, stop=True)
                    nc.tensor.matmul(pst2[:, SQ:2 * SQ], khB[:, :, kcol],
                                     qmB[:, :, :],
                                     perf_mode=DR, start=True, stop=True)
                    pt2 = ptp.tile([128, 2 * SQ], BF16, tag="pt")
                    nc.scalar.activation(pt2, pst2,
                                         mybir.ActivationFunctionType.Exp,
                                         scale=SCALE)
                    if kt == 0:
                        nc.vector.tensor_copy(den_e, pt2)
                    elif kt == 1:
                        nc.vector.tensor_copy(den_o, pt2)
                    elif kt % 2 == 0:
                        nc.vector.tensor_add(den_e, den_e, pt2)
                    else:
                        nc.vector.tensor_add(den_o, den_o, pt2)
                    vs = vt[kt]
                    nc.tensor.matmul(
                        potA, vs[:, colA], pt2[:, 0:SQ],
                        start=(kt == 0), stop=(kt == NST - 1))
                    nc.tensor.matmul(
                        potB, vs[:, colB], pt2[:, SQ:2 * SQ],
                        start=(kt == 0), stop=(kt == NST - 1))
                    if gen is not None:
                        for _ in range(2):
                            if next(gen, None) is None:
                                gen = None
                                break
                if gen is not None:
                    for _ in gen:
                        pass
                if pair == GH // 2 - 1:
                    state.pop(("v", g), None)
                    state.pop(("w", g), None)

                potA_s = otsp.tile([128, SQ], F32, tag="ots", name="potA_s")
                nc.scalar.copy(potA_s, potA)
                potB_s = otsp.tile([128, SQ], F32, tag="ots", name="potB_s")
                nc.scalar.copy(potB_s, potB)
                pending.append((hA, hB, potA_s, potB_s, den_e, den_o))

            d1s = _norm_den(nc, misc, st_ps, ones128, pending)
            _norm_fin(nc, misc, st_ps, oT, ones1, pending, d1s)
            pending.clear()

        if debug:
            for c in range(NKV):
                nc.sync.dma_start(
                    out=t["dbg_kvcT"][c * 128:(c + 1) * 128, :], in_=kvcT[c])
            for c in range(NQL):
                nc.sync.dma_start(
                    out=t["dbg_qcT"][c * 128:(c + 1) * 128, :], in_=qcT[c])
            nc.sync.dma_start(out=t["dbg_krT"][:, :], in_=krT)
            for h in range(H):
                nc.sync.dma_start(
                    out=t["dbg_oT"][h * 128:(h + 1) * 128, :], in_=oT[h])

        # ===================================================================
        # Phase B: output projection, all 16 heads, PSUM-accumulated.
        # h-outer over D-halves: wo streams as 16 [128, 1024] row tiles per
        # half (2KB contiguous DMA lines, one DMA per head) while the 8 PSUM
        # banks hold one half's accumulators; each stationary oT slice is
        # loaded once per (half, qt) and serves both 512-wide D blocks.
        # ===================================================================
        NQT = SQ // 128
        with nc.named_scope("pB_outproj", notify=True), \
             tc.tile_pool(name="wop", bufs=6) as wop, \
             tc.tile_pool(name="outs", bufs=4) as outs, \
             tc.tile_pool(name="po_ps", bufs=8, space="PSUM") as po_ps:
            for half in range(2):
                hcol = slice(half * 1024, (half + 1) * 1024)
                po = [[po_ps.tile([128, 512], F32, tag="po", name=f"po{b2}_{qt}")
                       for qt in range(NQT)] for b2 in range(2)]
                for h in range(H):
                    wo_t = wop.tile([128, 1024], BF16, tag="wo")
                    nc.sync.dma_start(
                        out=wo_t, in_=t["wo"][h * 128:(h + 1) * 128, hcol])
                    for qt in range(NQT):
                        for b2 in range(2):
                            nc.tensor.matmul(
                                po[b2][qt], oT[h][:, qt * 128:(qt + 1) * 128],
                                wo_t[:, b2 * 512:(b2 + 1) * 512],
                                start=(h == 0), stop=(h == H - 1))
                for b2 in range(2):
                    bcol = slice(half * 1024 + b2 * 512,
                                 half * 1024 + (b2 + 1) * 512)
                    for qt in range(NQT):
                        o_s = outs.tile([128, 512], F32, tag="os")
                        nc.scalar.copy(o_s, po[b2][qt])
                        nc.sync.dma_start(
                            out=t["out"][qt * 128:(qt + 1) * 128, bcol], in_=o_s)


_NC_CACHE = None


def _get_nc():
    global _NC_CACHE
    if _NC_CACHE is None:
        _NC_CACHE = build_nc()
    return _NC_CACHE


def _rope_tables(positions):
    """cos/sin tables in transposed-packed layout [128, len(positions)]:
    rows 0:64 and 64:128 both hold the [RD, s] table (two rope vectors are
    stacked per 128 partitions)."""
    inv_freq = 1.0 / (10000.0 ** (np.arange(0, RD, 2, dtype=np.float32) / RD))
    ang = positions[:, None].astype(np.float32) * inv_freq[None, :]  # [s, 32]
    cos = np.concatenate([np.cos(ang), np.cos(ang)], axis=-1)        # [s, 64]
    sin = np.concatenate([np.sin(ang), np.sin(ang)], axis=-1)
    cosT = np.ascontiguousarray(cos.T)                               # [64, s]
    sinT = np.ascontiguousarray(sin.T)
    return (np.concatenate([cosT, cosT], axis=0),
            np.concatenate([sinT, sinT], axis=0))


def _rot_perm():
    m = np.zeros((128, 128), dtype=np.float32)
    for b0 in (0, 64):
        for i in range(32):
            m[b0 + i + 32, b0 + i] = -1.0   # rot[m] = -t[m+32], m < 32
            m[b0 + i, b0 + i + 32] = 1.0    # rot[m] = +t[m-32], m >= 32
    return m


def kernel(x, Wcq, g_q, Wdq, Wdqr, Wckv, g_kv, Wdk, Wdv, Wkr, Wo):
    import ml_dtypes

    bf16 = ml_dtypes.bfloat16
    nc = _get_nc()

    x = np.asarray(x, dtype=np.float32)
    xT = [np.ascontiguousarray(x[b].T).astype(bf16) for b in range(B)]  # [D, S]
    wkr2 = np.ascontiguousarray(
        np.concatenate([Wkr, Wkr], axis=1)).astype(bf16)  # [D, 128]
    rotp = _rot_perm().astype(bf16)

    shared = {
        "wcq": np.ascontiguousarray(Wcq).astype(bf16),
        "wckv": np.ascontiguousarray(Wckv).astype(bf16),
        "wkr2": wkr2,
        "wdq": np.ascontiguousarray(Wdq).astype(bf16),
        "wdqr": np.ascontiguousarray(Wdqr).astype(bf16),
        "wdk": np.ascontiguousarray(Wdk).astype(bf16),
        "wdv": np.ascontiguousarray(Wdv).astype(bf16),
        "wo": np.ascontiguousarray(Wo).astype(bf16),
        "gqT": np.ascontiguousarray(
            np.asarray(g_q, dtype=np.float32).reshape(NQL, 128).T),
        "gkvT": np.ascontiguousarray(
            np.asarray(g_kv, dtype=np.float32).reshape(NKV, 128).T),
        "rotp": rotp,
    }

    # Each core sees the sequence rotated so its own query block sits at
    # columns 0:SQ (the SPMD program is position-independent; softmax over
    # keys is permutation invariant as long as the rope tables rotate too).
    in_maps = []
    for core in range(N_CORES):
        b, sl = core // 4, core % 4
        pos = np.roll(np.arange(S), -sl * SQ)
        ck, sk = _rope_tables(pos)
        m = dict(shared)
        m["xT"] = np.ascontiguousarray(np.roll(xT[b], -sl * SQ, axis=1))
        m["cosk"] = np.ascontiguousarray(ck)
        m["sink"] = np.ascontiguousarray(sk)
        in_maps.append(m)

    trace = bool(int(os.environ.get("MLA_TRACE", "0")))
    res = run_bass_kernel_spmd(
        nc, in_maps, core_ids=list(range(N_CORES)), trace=trace,
        trace_cores=list(range(N_CORES)) if trace else None,
        stitch_traces=bool(int(os.environ.get("MLA_STITCH", "0"))),
        tmpdir=os.environ.get("MLA_TMPDIR") or None,
    )
    kernel.last_result = res

    out = np.empty((B, S, D), dtype=np.float32)
    for core in range(N_CORES):
        b, sl = core // 4, core % 4
        out[b, sl * SQ:(sl + 1) * SQ, :] = res.results[core]["out"]
    return out

